# revision 34
# baseline (speedup 1.0000x reference)
"""Trainium2 Bass kernel for a 2-layer GAT (GNN message passing).

Strategy (8 NeuronCores, SPMD, single launch):
  - Destination-shard nodes: core c owns dst nodes [c*12500, (c+1)*12500).
    Each core receives all edges into its nodes -> segment softmax needs no
    cross-core reduction.
  - Node phase 1 on each core: h1 = x_slice @ [W1 | W1@A_s | W1@A_d] on PE,
    rows [h1(64) | alpha_s(8) | alpha_d(8)] stored as 256B bf16 rows in DRAM.
  - AllGather the row table (any core may need any src row).
  - Edge phase (bank-major): dma_gather fetches per-edge src rows (int16
    indices relative to one of 4 banks of <=32768 rows), 32 chunks (4096
    rows) per call to amortize the ~1us SWDGE fixed cost.  Per 128-edge
    chunk, segment aggregation is a PE matmul with a selection matrix built
    from an iota compare; alpha_dst is expanded per-edge with the transposed
    selection matrix (built by comparing iota against a PE-broadcast of the
    dst-row stream, read straight out of PSUM); softmax denominators ride
    along as extra matmul columns; the division is deferred to a per-node
    post-scale.
  - Node phase 2: divide, +bias, ELU, @ [W2 | folded attention vectors],
    AllGather, edge phase 2 (identical edge streams), log-softmax epilogue.
"""

import sys

sys.path.insert(0, "/opt/trn_rl_repo")

import numpy as np
import ml_dtypes

import concourse.bass as bass
import concourse.bacc as bacc
import concourse.mybir as mybir
from concourse.tile import TileContext
from concourse.bass_utils import run_bass_kernel_spmd

BF16 = ml_dtypes.bfloat16
P = 128
NCORES = 8

# ---------------------------------------------------------------- config


class Cfg:
    def __init__(self, n_nodes, n_edges, f_in, heads1, out1, n_classes,
                 npc, nbank, neg_slope=0.2):
        self.N = n_nodes
        self.E = n_edges
        self.F_IN = f_in                    # 256
        self.H1 = heads1                    # 8
        self.O1 = out1                      # 8
        self.C = n_classes                  # 40
        self.NEG = neg_slope
        self.NPC = npc                      # raw nodes per core
        assert npc * NCORES >= n_nodes
        self.TILES = (npc + P - 1) // P
        self.NPAD = self.TILES * P          # padded nodes per core
        self.NTOT = NCORES * self.NPAD      # table rows
        self.NBANK = nbank
        assert self.NTOT % nbank == 0
        self.BANK = self.NTOT // nbank
        assert self.BANK <= 32768
        self.BANK_ROWS = [self.BANK] * nbank
        self.BANK_BASE = [self.BANK * i for i in range(nbank)]
        self.D1 = heads1 * out1             # 64
        # layer-1 rows: 8 head-blocks of [8 msg | 1.0], then alpha_s(8), alpha_d(8)
        self.F1 = self.D1 + heads1          # 72 = interleaved msg+denom block
        self.WA1 = self.F1 + 2 * heads1     # 88 cols produced by node matmul 1
        # layer-2 rows: [40 msg | 1.0 | alpha_s | alpha_d]
        self.F2 = n_classes + 1             # 41 = msg+denom block
        self.WA2 = self.F2 + 2              # 43 cols produced by node matmul 2
        self.ROW1 = 128                     # bf16 elems/row in table1 (256B)
        self.ROW2 = 128                     # bf16 elems/row in table2 (256B)
        assert self.WA1 <= self.ROW1
        assert self.WA2 <= self.ROW2
        self.KCH = (f_in + P - 1) // P      # k-chunks in node matmul 1
        self.BATCH = 32                     # chunks per dma_gather call


FULL = Cfg(n_nodes=100000, n_edges=1600000, f_in=256, heads1=8, out1=8,
           n_classes=40, npc=12500, nbank=4)


# ------------------------------------------------------- host preprocessing


def build_edge_meta(cfg, src, dst):
    """Partition/sort/pad edges.  Returns (meta, per-core idx/drel streams).

    meta (identical across cores): per bank: chunk list [(tile, start, stop)],
    batches [(chunk_lo, chunk_hi)], plus global chunk offsets per bank.
    """
    src_row = (src // cfg.NPC) * cfg.NPAD + (src % cfg.NPC)
    dst_core = dst // cfg.NPC
    dst_loc = dst % cfg.NPC
    tile = dst_loc // P
    drel = dst_loc % P
    bank = src_row // cfg.BANK
    bidx = src_row % cfg.BANK

    counts = np.zeros((NCORES, cfg.NBANK, cfg.TILES), np.int64)
    np.add.at(counts, (dst_core, bank, tile), 1)
    K = np.ceil(counts.max(axis=0) / P).astype(np.int64)      # [NBANK, TILES]

    # pad each bank's chunk count to a multiple of 4 (group granularity)
    for b in range(cfg.NBANK):
        tot = int(K[b].sum())
        extra = (-tot) % 4
        if extra and tot > 0:
            tstar = int(np.nonzero(K[b])[0][-1])
            K[b, tstar] += extra

    chunks = []          # per bank: list of (tile, start, stop)
    batches = []         # per bank: list of (lo, hi)
    for b in range(cfg.NBANK):
        ch = []
        for t in range(cfg.TILES):
            k = int(K[b, t])
            for i in range(k):
                ch.append((t, i == 0, i == k - 1))
        chunks.append(ch)
        bt = []
        lo = 0
        while lo < len(ch):
            hi = min(lo + cfg.BATCH, len(ch))
            bt.append((lo, hi))
            lo = hi
        batches.append(bt)

    nch_bank = [len(c) for c in chunks]
    nch_tot = sum(nch_bank)
    bank_off = np.cumsum([0] + nch_bank)[:-1]

    order_key = (dst_core * cfg.NBANK + bank) * cfg.TILES + tile
    perm = np.argsort(order_key, kind="stable")
    s_core, s_bank, s_tile = dst_core[perm], bank[perm], tile[perm]
    s_bidx, s_drel = bidx[perm], drel[perm]

    gidx_all = np.zeros((NCORES, nch_tot * P), np.int16)
    drel_all = np.full((NCORES, nch_tot * P), -1.0, np.float32)

    run_off = np.zeros((NCORES, cfg.NBANK, cfg.TILES), np.int64)
    for b in range(cfg.NBANK):
        off = 0
        for t in range(cfg.TILES):
            run_off[:, b, t] = bank_off[b] * P + off * P
            off += int(K[b, t])
    grp = s_core * (cfg.NBANK * cfg.TILES) + s_bank * cfg.TILES + s_tile
    first = np.r_[True, grp[1:] != grp[:-1]]
    gstart = np.maximum.accumulate(np.where(first, np.arange(len(grp)), 0))
    within = np.arange(len(grp)) - gstart
    pos = run_off[s_core, s_bank, s_tile] + within
    gidx_all[s_core, pos] = s_bidx.astype(np.int16)
    drel_all[s_core, pos] = s_drel.astype(np.float32)

    meta = dict(K=K, chunks=chunks, batches=batches, bank_off=bank_off,
                nch_tot=nch_tot)
    return meta, gidx_all, drel_all


def wrap_idx(gidx_flat):
    """idx stream [E] -> dma_gather layout [128, E/16] (16-lane wrap,
    replicated into the 8 sixteen-partition groups)."""
    e = gidx_flat.shape[0]
    assert e % 16 == 0
    w = gidx_flat.reshape(e // 16, 16).T          # [16, E/16]
    return np.tile(w, (8, 1)).astype(np.int16)     # [128, E/16]


def prep_core_inputs(cfg, meta, core, x, W1, a_s1, a_d1, b1, W2, a_s2, a_d2,
                     b2, gidx_core, drel_core):
    n0, n1 = core * cfg.NPC, min((core + 1) * cfg.NPC, cfg.N)
    xs = np.zeros((cfg.NPAD, cfg.F_IN), np.float32)
    xs[: n1 - n0] = x[n0:n1]
    xT = np.ascontiguousarray(xs.T)                          # [F_IN, NPAD]
    kch = cfg.KCH
    xT_s = np.zeros((kch, P, cfg.NPAD), np.float32)
    for k in range(kch):
        lo, hi = k * P, min((k + 1) * P, cfg.F_IN)
        xT_s[k, : hi - lo] = xT[lo:hi]
    # [P, KCH, NPAD] so each node tile loads with ONE dma
    xT_s = np.ascontiguousarray(xT_s.transpose(1, 0, 2)).astype(BF16)

    A_s = np.zeros((cfg.D1, cfg.H1), np.float32)
    A_d = np.zeros((cfg.D1, cfg.H1), np.float32)
    for h in range(cfg.H1):
        A_s[h * cfg.O1:(h + 1) * cfg.O1, h] = a_s1[h]
        A_d[h * cfg.O1:(h + 1) * cfg.O1, h] = a_d1[h]
    # interleave: per head [8 msg cols | 1 zero col (1.0 memset later)]
    Wmsg = np.zeros((cfg.F_IN, cfg.F1), np.float32)
    for h in range(cfg.H1):
        Wmsg[:, h * (cfg.O1 + 1):h * (cfg.O1 + 1) + cfg.O1] = \
            W1[:, h * cfg.O1:(h + 1) * cfg.O1]
    Wfull = np.concatenate([Wmsg, W1 @ A_s, W1 @ A_d], axis=1)  # [F_IN, 88]
    wall = np.zeros((kch, P, cfg.WA1), np.float32)
    for k in range(kch):
        lo, hi = k * P, min((k + 1) * P, cfg.F_IN)
        wall[k, : hi - lo] = Wfull[lo:hi]
    wall = wall.astype(BF16)

    # layer 2: [W2 | zero col | W2@a_s2 | W2@a_d2]  (43 cols)
    w2aug = np.concatenate(
        [W2, np.zeros((cfg.D1, 1), np.float32),
         (W2 @ a_s2[0])[:, None], (W2 @ a_d2[0])[:, None]], axis=1
    ).astype(np.float32)

    bias1r = np.tile(b1[None, :], (P, 1)).astype(np.float32)
    bias2r = np.tile(b2[None, :], (P, 1)).astype(np.float32)
    iotar = np.tile(np.arange(P, dtype=np.float32)[None, :], (P, 1)).astype(BF16)
    iotap = np.arange(P, dtype=np.float32)[:, None].copy()
    ones1 = np.ones((1, P), BF16)
    identm = np.eye(P, dtype=np.float32)

    nch = meta["nch_tot"]
    gidx = wrap_idx(gidx_core)                               # [128, nch*8]
    drelc = np.ascontiguousarray(
        drel_core.reshape(nch, P).T).astype(np.float32)      # [128, nch]
    rows = []
    for b in range(cfg.NBANK):
        off = meta["bank_off"][b]
        for (lo, hi) in meta["batches"][b]:
            r = np.full((1, cfg.BATCH * P), -1.0, np.float32)
            r[0, : (hi - lo) * P] = drel_core[(off + lo) * P:(off + hi) * P]
            rows.append(r)
    drelf = (np.stack(rows).astype(BF16) if rows
             else np.zeros((1, 1, cfg.BATCH * P), BF16))

    return dict(xT=xT_s, wall=wall, w2aug=w2aug, bias1r=bias1r, bias2r=bias2r,
                iotar=iotar, iotap=iotap, ones1=ones1, identd=identm,
                gidx=gidx, drelc=drelc, drelf=drelf)


# ------------------------------------------------------------ bass program


def build_program(cfg, meta, phases="ABCDEFG"):
    nc = bacc.Bacc(None, target_bir_lowering=False, debug=False)
    f32, bf16, i16 = mybir.dt.float32, mybir.dt.bfloat16, mybir.dt.int16

    nch = meta["nch_tot"]
    nbatch_tot = sum(len(b) for b in meta["batches"])

    xT = nc.declare_dram_parameter("xT", [P, cfg.KCH, cfg.NPAD], bf16, isOutput=False)
    wall = nc.declare_dram_parameter("wall", [cfg.KCH, P, cfg.WA1], bf16, isOutput=False)
    w2aug = nc.declare_dram_parameter("w2aug", [cfg.D1, cfg.WA2], f32, isOutput=False)
    bias1r = nc.declare_dram_parameter("bias1r", [P, cfg.D1], f32, isOutput=False)
    bias2r = nc.declare_dram_parameter("bias2r", [P, cfg.C], f32, isOutput=False)
    identd = nc.declare_dram_parameter("identd", [P, P], f32, isOutput=False)
    iotar_d = nc.declare_dram_parameter("iotar", [P, P], bf16, isOutput=False)
    iotap_d = nc.declare_dram_parameter("iotap", [P, 1], f32, isOutput=False)
    ones1_d = nc.declare_dram_parameter("ones1", [1, P], bf16, isOutput=False)
    gidx_d = nc.declare_dram_parameter("gidx", [P, nch * 8], i16, isOutput=False)
    drelc_d = nc.declare_dram_parameter("drelc", [P, nch], f32, isOutput=False)
    drelf_d = nc.declare_dram_parameter("drelf", [nbatch_tot, 1, cfg.BATCH * P], bf16, isOutput=False)
    out_d = nc.declare_dram_parameter("out", [cfg.NPAD, cfg.C], f32, isOutput=True)

    t1loc = nc.dram_tensor("t1loc", [cfg.NPAD, cfg.ROW1], bf16)
    t1full = nc.dram_tensor("t1full", [cfg.NTOT, cfg.ROW1], bf16, addr_space="Shared")
    t2loc = nc.dram_tensor("t2loc", [cfg.NPAD, cfg.ROW2], bf16)
    t2full = nc.dram_tensor("t2full", [cfg.NTOT, cfg.ROW2], bf16, addr_space="Shared")

    H1, D1, O1, C = cfg.H1, cfg.D1, cfg.O1, cfg.C
    F1, F2 = cfg.F1, cfg.F2

    with TileContext(nc) as tc:
        with tc.tile_pool(name="persist", bufs=1) as pp:
            ident = pp.tile([P, P], f32)
            nc.sync.dma_start(out=ident[:], in_=identd[:])
            wall_sb = pp.tile([P, cfg.KCH, cfg.WA1], bf16)
            for k in range(cfg.KCH):
                nc.sync.dma_start(out=wall_sb[:, k, :], in_=wall[k])
            w2aug_sb = pp.tile([D1, cfg.WA2], f32)
            nc.sync.dma_start(out=w2aug_sb[:], in_=w2aug[:])
            b1_sb = pp.tile([P, D1], f32)
            nc.sync.dma_start(out=b1_sb[:], in_=bias1r[:])
            b2_sb = pp.tile([P, C], f32)
            nc.sync.dma_start(out=b2_sb[:], in_=bias2r[:])
            iotar_sb = pp.tile([P, P], bf16)
            nc.sync.dma_start(out=iotar_sb[:], in_=iotar_d[:])
            iotap_sb = pp.tile([P, 1], f32)
            nc.sync.dma_start(out=iotap_sb[:], in_=iotap_d[:])
            ones1_sb = pp.tile([1, P], bf16)
            nc.sync.dma_start(out=ones1_sb[:], in_=ones1_d[:])
            ad1_buf = pp.tile([P, cfg.TILES * H1], bf16)
            ad2_buf = pp.tile([P, cfg.TILES], bf16)
            agg1 = pp.tile([P, cfg.TILES * F1], f32)
            agg2 = pp.tile([P, cfg.TILES * F2], f32)
            ssum_all = pp.tile([P, cfg.TILES], f32)
            lns_all = pp.tile([P, cfg.TILES], f32)
            scr_g = pp.tile([1, cfg.ROW1], bf16)

            # ---------------- phase A: node transform layer 1
            if "A" in phases:
                with nc.named_scope("A_node1"):
                    node_phase1(nc, tc, cfg, xT, wall_sb, ident, ad1_buf, t1loc)

            # ---------------- phase B: allgather table 1
            # (single large AllGather: the collective cost is dominated by a
            # fixed small-transfer bandwidth floor, so splitting loses more
            # than bank-level overlap would gain)
            if "B" in phases:
                with nc.named_scope("B_ag1"):
                    nc.gpsimd.collective_compute(
                        "AllGather", mybir.AluOpType.bypass,
                        replica_groups=[list(range(NCORES))],
                        ins=[t1loc[:]], outs=[t1full[:]])

            # ---------------- phase C: edge layer 1
            if "C" in phases:
                with nc.named_scope("C_edge1"):
                    nc.vector.memset(agg1[:], 1e-16)
                    edge_phase(nc, tc, cfg, meta, layer=1, table=t1full,
                               row_elems=cfg.ROW1, fcols=F1, gidx_d=gidx_d,
                               drelc_d=drelc_d, drelf_d=drelf_d,
                               iotar_sb=iotar_sb, iotap_sb=iotap_sb,
                               ones1_sb=ones1_sb, scr_g=scr_g,
                               ad_buf=ad1_buf, agg=agg1)

            # ---------------- phase D: node transform layer 2
            if "D" in phases:
                with nc.named_scope("D_node2"):
                    node_phase2(nc, tc, cfg, agg1, b1_sb, w2aug_sb, ident,
                                ad2_buf, t2loc)

            # ---------------- phase E: allgather table 2
            if "E" in phases:
                with nc.named_scope("E_ag2"):
                    nc.gpsimd.collective_compute(
                        "AllGather", mybir.AluOpType.bypass,
                        replica_groups=[list(range(NCORES))],
                        ins=[t2loc[:]], outs=[t2full[:]])

            # ---------------- phase F: edge layer 2
            if "F" in phases:
                with nc.named_scope("F_edge2"):
                    nc.vector.memset(agg2[:], 1e-16)
                    edge_phase(nc, tc, cfg, meta, layer=2, table=t2full,
                               row_elems=cfg.ROW2, fcols=F2, gidx_d=gidx_d,
                               drelc_d=drelc_d, drelf_d=drelf_d,
                               iotar_sb=iotar_sb, iotap_sb=iotap_sb,
                               ones1_sb=ones1_sb, scr_g=scr_g,
                               ad_buf=ad2_buf, agg=agg2)

            # ---------------- phase G: epilogue (divide, bias, log_softmax)
            if "G" in phases:
                with nc.named_scope("G_epilogue"):
                    epilogue(nc, tc, cfg, agg2, b2_sb, out_d, ssum_all, lns_all)
            else:
                with tc.tile_pool(name="zz", bufs=1) as zz:
                    z = zz.tile([P, cfg.C], f32)
                    nc.vector.memset(z[:], 0.0)
                    for t in range(cfg.TILES):
                        nc.sync.dma_start(out=out_d[t * P:(t + 1) * P, :],
                                          in_=z[:])

    nc.compile()
    return nc


def node_phase1(nc, tc, cfg, xT, wall_sb, ident, ad1_buf, t1loc):
    f32, bf16 = mybir.dt.float32, mybir.dt.bfloat16
    H1, D1, F1, WA = cfg.H1, cfg.D1, cfg.F1, cfg.WA1
    with tc.tile_pool(name="na", bufs=3) as na, \
         tc.tile_pool(name="napsum", bufs=2, space="PSUM") as nap:
        for t in range(cfg.TILES):
            xt = na.tile([P, cfg.KCH, P], bf16, tag="xt")
            nc.sync.dma_start(out=xt[:], in_=xT[:, :, t * P:(t + 1) * P])
            row = na.tile([P, cfg.ROW1], bf16, tag="row")
            ph = nap.tile([WA, P], f32, tag="ph")
            for k in range(cfg.KCH):
                nc.tensor.matmul(out=ph[:], lhsT=wall_sb[:, k, :],
                                 rhs=xt[:, k, :],
                                 start=(k == 0), stop=(k == cfg.KCH - 1))
            hT = na.tile([WA, P], f32, tag="hT")
            nc.scalar.copy(out=hT[:], in_=ph[:])
            pr = nap.tile([P, WA], f32, tag="pr")
            nc.tensor.transpose(out=pr[:], in_=hT[:],
                                identity=ident[:WA, :WA])
            nc.vector.memset(row[:, WA:], 0.0)
            nc.scalar.copy(out=row[:, :WA], in_=pr[:])
            # denominator ride-along columns (head-interleaved col 8 of 9)
            nc.vector.memset(
                row[:, :F1].rearrange(
                    "p (h q) -> p h q", q=cfg.O1 + 1)[:, :, cfg.O1:], 1.0)
            nc.vector.tensor_copy(
                out=ad1_buf[:, t * H1:(t + 1) * H1],
                in_=pr[:, F1 + H1:F1 + 2 * H1])
            nc.scalar.dma_start(out=t1loc[t * P:(t + 1) * P, :], in_=row[:])


def node_phase2(nc, tc, cfg, agg1, b1_sb, w2aug_sb, ident, ad2_buf, t2loc):
    f32, bf16 = mybir.dt.float32, mybir.dt.bfloat16
    H1, D1, O1, C, F1, WA2 = cfg.H1, cfg.D1, cfg.O1, cfg.C, cfg.F1, cfg.WA2
    Q = O1 + 1
    with tc.tile_pool(name="nb", bufs=3) as nb, \
         tc.tile_pool(name="nbpsum", bufs=2, space="PSUM") as nbp:
        for t in range(cfg.TILES):
            a_t = agg1[:, t * F1:(t + 1) * F1].rearrange(
                "p (h q) -> p h q", q=Q)
            rec = nb.tile([P, H1], f32, tag="rec")
            nc.vector.reciprocal(
                out=rec[:].unsqueeze(2),
                in_=a_t[:, :, O1:])
            o1 = nb.tile([P, D1], f32, tag="o1")
            nc.vector.tensor_tensor(
                out=o1[:].rearrange("p (h o) -> p h o", h=H1),
                in0=a_t[:, :, :O1],
                in1=rec[:].unsqueeze(2).to_broadcast([P, H1, O1]),
                op=mybir.AluOpType.mult)
            nc.vector.tensor_add(out=o1[:], in0=o1[:], in1=b1_sb[:])
            # elu
            eneg = nb.tile([P, D1], f32, tag="eneg")
            nc.vector.tensor_scalar_min(eneg[:], o1[:], 0.0)
            nc.scalar.activation(out=eneg[:], in_=eneg[:],
                                 func=mybir.ActivationFunctionType.Exp)
            h = nb.tile([P, D1], f32, tag="h")
            nc.vector.tensor_scalar_max(h[:], o1[:], 0.0)
            nc.vector.tensor_add(out=h[:], in0=h[:], in1=eneg[:])
            nc.vector.tensor_scalar_add(h[:], h[:], -1.0)
            # h2 = [elu] @ w2aug via two PE transposes
            phT = nbp.tile([D1, P], f32, tag="phT")
            nc.tensor.transpose(out=phT[:], in_=h[:], identity=ident[:])
            hT2 = nb.tile([D1, P], f32, tag="hT2")
            nc.scalar.copy(out=hT2[:], in_=phT[:])
            p2T = nbp.tile([WA2, P], f32, tag="p2T")
            nc.tensor.matmul(out=p2T[:], lhsT=w2aug_sb[:], rhs=hT2[:],
                             start=True, stop=True)
            h2T = nb.tile([WA2, P], f32, tag="h2T")
            nc.scalar.copy(out=h2T[:], in_=p2T[:])
            p2 = nbp.tile([P, WA2], f32, tag="p2")
            nc.tensor.transpose(out=p2[:], in_=h2T[:],
                                identity=ident[:WA2, :WA2])
            row2 = nb.tile([P, cfg.ROW2], bf16, tag="row2")
            nc.vector.memset(row2[:, WA2:], 0.0)
            nc.scalar.copy(out=row2[:, :WA2], in_=p2[:])
            nc.vector.memset(row2[:, C:C + 1], 1.0)   # denominator column
            nc.vector.tensor_copy(out=ad2_buf[:, t:t + 1],
                                  in_=p2[:, WA2 - 1:WA2])
            nc.scalar.dma_start(out=t2loc[t * P:(t + 1) * P, :], in_=row2[:])


def epilogue(nc, tc, cfg, agg2, b2_sb, out_d, ssum_all, lns_all):
    f32 = mybir.dt.float32
    C, F2 = cfg.C, cfg.F2
    with tc.tile_pool(name="ep", bufs=3) as ep:
        # pass 1: o2 = agg2_msg * (1/denom) + bias, written in place into
        # agg2's msg columns; exp(o2) only to accumulate the softmax sums
        for t in range(cfg.TILES):
            rec = ep.tile([P, 1], f32, tag="rec2")
            nc.vector.reciprocal(
                out=rec[:], in_=agg2[:, t * F2 + C:t * F2 + C + 1])
            o2 = agg2[:, t * F2:t * F2 + C]
            nc.vector.scalar_tensor_tensor(
                out=o2, in0=o2, scalar=rec[:], in1=b2_sb[:],
                op0=mybir.AluOpType.mult, op1=mybir.AluOpType.add)
            exps = ep.tile([P, C], f32, tag="exps")
            nc.scalar.activation(out=exps[:], in_=o2,
                                 func=mybir.ActivationFunctionType.Exp,
                                 accum_out=ssum_all[:, t:t + 1])
        # pass 2: one Ln over all tiles, then subtract + store
        nc.scalar.activation(out=lns_all[:], in_=ssum_all[:],
                             func=mybir.ActivationFunctionType.Ln)
        for t in range(cfg.TILES):
            fin = ep.tile([P, C], f32, tag="fin")
            nc.vector.tensor_tensor(
                out=fin[:], in0=agg2[:, t * F2:t * F2 + C],
                in1=lns_all[:, t:t + 1].to_broadcast([P, C]),
                op=mybir.AluOpType.subtract)
            nc.scalar.dma_start(out=out_d[t * P:(t + 1) * P, :], in_=fin[:])


def edge_phase(nc, tc, cfg, meta, layer, table, row_elems, fcols, gidx_d,
               drelc_d, drelf_d, iotar_sb, iotap_sb, ones1_sb, scr_g,
               ad_buf, agg):
    f32, bf16, i16 = mybir.dt.float32, mybir.dt.bfloat16, mybir.dt.int16
    H = cfg.H1 if layer == 1 else 1
    Q = (cfg.O1 + 1) if layer == 1 else cfg.F2   # per-head block incl denom
    asl_lo = fcols                               # alpha_src col within row
    B = cfg.BATCH

    with tc.tile_pool(name=f"eg{layer}", bufs=3) as eg, \
         tc.tile_pool(name=f"es{layer}", bufs=2) as es, \
         tc.tile_pool(name=f"em{layer}", bufs=3) as em, \
         tc.tile_pool(name=f"epr{layer}", bufs=2, space="PSUM") as epr, \
         tc.tile_pool(name=f"epa{layer}", bufs=2, space="PSUM") as epa, \
         tc.tile_pool(name=f"epd{layer}", bufs=2, space="PSUM") as epd:
        psum_agg = None
        nb_seen = 0
        for b in range(cfg.NBANK):
            off = int(meta["bank_off"][b])
            tbl_bank = table[cfg.BANK_BASE[b]:
                             cfg.BANK_BASE[b] + cfg.BANK_ROWS[b], :]
            # gate this bank's gathers on its AllGather slice (Pool runs
            # in order, so later banks' gathers don't wait on earlier ones)
            nc.gpsimd.dma_start(out=scr_g[:], in_=tbl_bank[0:1, :])
            for (lo, hi) in meta["batches"][b]:
                nchb = hi - lo
                ngrp = nchb // 4
                idx_t = em.tile([P, B * 8], i16, tag="idx")
                nc.sync.dma_start(
                    out=idx_t[:, :nchb * 8],
                    in_=gidx_d[:, (off + lo) * 8:(off + hi) * 8])
                drc_t = em.tile([P, B], f32, tag="drc")
                nc.sync.dma_start(out=drc_t[:, :nchb],
                                  in_=drelc_d[:, off + lo:off + hi])
                drf_t = em.tile([1, B * P], bf16, tag="drf")
                nc.sync.dma_start(out=drf_t[:], in_=drelf_d[nb_seen])
                g = eg.tile([P, B, row_elems], bf16, tag="g")
                nc.gpsimd.memset(g[0:1, 0:1, 0:4], 0.0)   # hoist WAR dep
                nc.gpsimd.dma_gather(
                    out_ap=g[:, :nchb, :], in_ap=tbl_bank,
                    idxs_ap=idx_t[:, :nchb * 8], num_idxs=nchb * P,
                    num_idxs_reg=nchb * P, elem_size=row_elems,
                    single_packet=False)
                nb_seen += 1

                # per-batch tiles: scatter masks + edge weights
                st_all = es.tile([P, B, P], bf16, tag="st")
                w_all = em.tile([P, B, H], f32, tag="w")
                wmx = em.tile([P, B, H], f32, tag="wmx")
                pad_all = epd.tile([P, B * H], f32, tag="pad")

                for gi in range(ngrp):
                    c0 = 4 * gi
                    pr1 = epr.tile([P, 512], f32, tag="pr1")
                    nc.tensor.matmul(out=pr1[:], lhsT=ones1_sb[:],
                                     rhs=drf_t[0:1, gi * 512:(gi + 1) * 512],
                                     start=True, stop=True)
                    spre = em.tile([P, 512], bf16, tag="spre")
                    nc.scalar.copy(out=spre[:], in_=pr1[:])
                    s = em.tile([P, 4, P], bf16, tag="s")
                    nc.vector.tensor_scalar(
                        out=s[:].rearrange("p a b -> p (a b)"),
                        in0=spre[:], scalar1=iotap_sb[:], scalar2=None,
                        op0=mybir.AluOpType.is_equal)
                    for c in range(4):
                        nc.vector.tensor_scalar(
                            out=st_all[:, c0 + c, :], in0=iotar_sb[:],
                            scalar1=drc_t[:, c0 + c:c0 + c + 1], scalar2=None,
                            op0=mybir.AluOpType.is_equal)
                        t_c = meta["chunks"][b][lo + c0 + c][0]
                        nc.tensor.matmul(
                            out=pad_all[:, (c0 + c) * H:(c0 + c + 1) * H],
                            lhsT=s[:, c, :],
                            rhs=ad_buf[:, t_c * H:(t_c + 1) * H],
                            start=True, stop=True)

                # batched edge-weight pipeline: e = as + ad; exp(leakyrelu(e))
                nc.vector.tensor_tensor(
                    out=w_all[:, :nchb, :],
                    in0=g[:, :nchb, asl_lo:asl_lo + H],
                    in1=pad_all[:, :nchb * H].rearrange(
                        "p (a h) -> p a h", h=H),
                    op=mybir.AluOpType.add)
                nc.vector.scalar_tensor_tensor(
                    out=wmx[:, :nchb, :], in0=w_all[:, :nchb, :],
                    scalar=cfg.NEG, in1=w_all[:, :nchb, :],
                    op0=mybir.AluOpType.mult, op1=mybir.AluOpType.max)
                nc.scalar.activation(
                    out=w_all[:, :nchb, :], in_=wmx[:, :nchb, :],
                    func=mybir.ActivationFunctionType.Exp)

                for gi in range(ngrp):
                    c0 = 4 * gi
                    m = em.tile([P, 4, fcols], bf16, tag="m")
                    nc.vector.tensor_tensor(
                        out=m[:].rearrange("p a (h q) -> p a h q", q=Q),
                        in0=g[:, c0:c0 + 4, :fcols].rearrange(
                            "p a (h q) -> p a h q", q=Q),
                        in1=w_all[:, c0:c0 + 4, :].unsqueeze(3).to_broadcast(
                            [P, 4, H, Q]),
                        op=mybir.AluOpType.mult)
                    for c in range(4):
                        t_c, start_c, stop_c = meta["chunks"][b][lo + c0 + c]
                        if start_c:
                            psum_agg = epa.tile([P, fcols], f32, tag="agg")
                        nc.tensor.matmul(out=psum_agg[:],
                                         lhsT=st_all[:, c0 + c, :],
                                         rhs=m[:, c, :],
                                         start=start_c, stop=stop_c)
                        if stop_c:
                            nc.vector.tensor_tensor(
                                out=agg[:, t_c * fcols:(t_c + 1) * fcols],
                                in0=agg[:, t_c * fcols:(t_c + 1) * fcols],
                                in1=psum_agg[:], op=mybir.AluOpType.add)


# ------------------------------------------------------------------ kernel

_CACHE = {}


def kernel(**inputs):
    cfg = FULL
    x = np.asarray(inputs["x"], np.float32)
    ei = np.asarray(inputs["edge_index"])
    W1 = np.asarray(inputs["W1"], np.float32)
    a_s1 = np.asarray(inputs["att_src1"], np.float32)
    a_d1 = np.asarray(inputs["att_dst1"], np.float32)
    b1 = np.asarray(inputs["bias1"], np.float32)
    W2 = np.asarray(inputs["W2"], np.float32)
    a_s2 = np.asarray(inputs["att_src2"], np.float32)
    a_d2 = np.asarray(inputs["att_dst2"], np.float32)
    b2 = np.asarray(inputs["bias2"], np.float32)

    loops = np.arange(cfg.N, dtype=np.int64)
    src = np.concatenate([ei[0].astype(np.int64), loops])
    dst = np.concatenate([ei[1].astype(np.int64), loops])

    meta, gidx_all, drel_all = build_edge_meta(cfg, src, dst)

    schedule_sig = tuple(
        tuple(meta["chunks"][b]) for b in range(cfg.NBANK))
    key = ("full", meta["nch_tot"], hash(schedule_sig))
    if key not in _CACHE:
        _CACHE[key] = build_program(cfg, meta)
    nc = _CACHE[key]

    in_maps = []
    for c in range(NCORES):
        in_maps.append(prep_core_inputs(
            cfg, meta, c, x, W1, a_s1, a_d1, b1, W2, a_s2, a_d2, b2,
            gidx_all[c], drel_all[c]))
    res = run_bass_kernel_spmd(nc, in_maps, list(range(NCORES)))
    outs = [res.results[c]["out"][: cfg.NPC] for c in range(NCORES)]
    return np.concatenate(outs, axis=0)[: cfg.N].astype(np.float32)


# revision 36
# speedup vs baseline: 1.0126x; 1.0126x over previous
"""Trainium2 Bass kernel for a 2-layer GAT (GNN message passing).

Strategy (8 NeuronCores, SPMD, single launch):
  - Destination-shard nodes: core c owns dst nodes [c*12500, (c+1)*12500).
    Each core receives all edges into its nodes -> segment softmax needs no
    cross-core reduction.
  - Node phase 1 on each core: h1 = x_slice @ [W1 | W1@A_s | W1@A_d] on PE,
    rows [h1(64) | alpha_s(8) | alpha_d(8)] stored as 256B bf16 rows in DRAM.
  - AllGather the row table (any core may need any src row).
  - Edge phase (bank-major): dma_gather fetches per-edge src rows (int16
    indices relative to one of 4 banks of <=32768 rows), 32 chunks (4096
    rows) per call to amortize the ~1us SWDGE fixed cost.  Per 128-edge
    chunk, segment aggregation is a PE matmul with a selection matrix built
    from an iota compare; alpha_dst is expanded per-edge with the transposed
    selection matrix (built by comparing iota against a PE-broadcast of the
    dst-row stream, read straight out of PSUM); softmax denominators ride
    along as extra matmul columns; the division is deferred to a per-node
    post-scale.
  - Node phase 2: divide, +bias, ELU, @ [W2 | folded attention vectors],
    AllGather, edge phase 2 (identical edge streams), log-softmax epilogue.
"""

import sys

sys.path.insert(0, "/opt/trn_rl_repo")

import numpy as np
import ml_dtypes

import concourse.bass as bass
import concourse.bacc as bacc
import concourse.mybir as mybir
from concourse.tile import TileContext
from concourse.bass_utils import run_bass_kernel_spmd

BF16 = ml_dtypes.bfloat16
P = 128
NCORES = 8

# ---------------------------------------------------------------- config


class Cfg:
    def __init__(self, n_nodes, n_edges, f_in, heads1, out1, n_classes,
                 npc, nbank, neg_slope=0.2):
        self.N = n_nodes
        self.E = n_edges
        self.F_IN = f_in                    # 256
        self.H1 = heads1                    # 8
        self.O1 = out1                      # 8
        self.C = n_classes                  # 40
        self.NEG = neg_slope
        self.NPC = npc                      # raw nodes per core
        assert npc * NCORES >= n_nodes
        self.TILES = (npc + P - 1) // P
        self.NPAD = self.TILES * P          # padded nodes per core
        self.NTOT = NCORES * self.NPAD      # table rows
        self.NBANK = nbank
        assert self.NTOT % nbank == 0
        self.BANK = self.NTOT // nbank
        assert self.BANK <= 32768
        self.BANK_ROWS = [self.BANK] * nbank
        self.BANK_BASE = [self.BANK * i for i in range(nbank)]
        self.D1 = heads1 * out1             # 64
        # layer-1 rows: 8 head-blocks of [8 msg | 1.0], then alpha_s(8), alpha_d(8)
        self.F1 = self.D1 + heads1          # 72 = interleaved msg+denom block
        self.WA1 = self.F1 + 2 * heads1     # 88 cols produced by node matmul 1
        # layer-2 rows: [40 msg | 1.0 | alpha_s | alpha_d]
        self.F2 = n_classes + 1             # 41 = msg+denom block
        self.WA2 = self.F2 + 2              # 43 cols produced by node matmul 2
        self.ROW1 = 128                     # bf16 elems/row in table1 (256B)
        self.ROW2 = 128                     # bf16 elems/row in table2 (256B)
        assert self.WA1 <= self.ROW1
        assert self.WA2 <= self.ROW2
        self.KCH = (f_in + P - 1) // P      # k-chunks in node matmul 1
        self.BATCH = 32                     # chunks per dma_gather call


FULL = Cfg(n_nodes=100000, n_edges=1600000, f_in=256, heads1=8, out1=8,
           n_classes=40, npc=12500, nbank=4)


# ------------------------------------------------------- host preprocessing


def build_edge_meta(cfg, src, dst):
    """Partition/sort/pad edges.  Returns (meta, per-core idx/drel streams).

    meta (identical across cores): per bank: chunk list [(tile, start, stop)],
    batches [(chunk_lo, chunk_hi)], plus global chunk offsets per bank.
    """
    src_row = (src // cfg.NPC) * cfg.NPAD + (src % cfg.NPC)
    dst_core = dst // cfg.NPC
    dst_loc = dst % cfg.NPC
    tile = dst_loc // P
    drel = dst_loc % P
    bank = src_row // cfg.BANK
    bidx = src_row % cfg.BANK

    counts = np.zeros((NCORES, cfg.NBANK, cfg.TILES), np.int64)
    np.add.at(counts, (dst_core, bank, tile), 1)
    K = np.ceil(counts.max(axis=0) / P).astype(np.int64)      # [NBANK, TILES]

    # pad each bank's chunk count to a multiple of 4 (group granularity)
    for b in range(cfg.NBANK):
        tot = int(K[b].sum())
        extra = (-tot) % 4
        if extra and tot > 0:
            tstar = int(np.nonzero(K[b])[0][-1])
            K[b, tstar] += extra

    chunks = []          # per bank: list of (tile, start, stop)
    batches = []         # per bank: list of (lo, hi)
    for b in range(cfg.NBANK):
        ch = []
        for t in range(cfg.TILES):
            k = int(K[b, t])
            for i in range(k):
                ch.append((t, i == 0, i == k - 1))
        chunks.append(ch)
        bt = []
        lo = 0
        while lo < len(ch):
            hi = min(lo + cfg.BATCH, len(ch))
            bt.append((lo, hi))
            lo = hi
        batches.append(bt)

    nch_bank = [len(c) for c in chunks]
    nch_tot = sum(nch_bank)
    bank_off = np.cumsum([0] + nch_bank)[:-1]

    order_key = (dst_core * cfg.NBANK + bank) * cfg.TILES + tile
    perm = np.argsort(order_key, kind="stable")
    s_core, s_bank, s_tile = dst_core[perm], bank[perm], tile[perm]
    s_bidx, s_drel = bidx[perm], drel[perm]

    gidx_all = np.zeros((NCORES, nch_tot * P), np.int16)
    drel_all = np.full((NCORES, nch_tot * P), -1.0, np.float32)

    run_off = np.zeros((NCORES, cfg.NBANK, cfg.TILES), np.int64)
    for b in range(cfg.NBANK):
        off = 0
        for t in range(cfg.TILES):
            run_off[:, b, t] = bank_off[b] * P + off * P
            off += int(K[b, t])
    grp = s_core * (cfg.NBANK * cfg.TILES) + s_bank * cfg.TILES + s_tile
    first = np.r_[True, grp[1:] != grp[:-1]]
    gstart = np.maximum.accumulate(np.where(first, np.arange(len(grp)), 0))
    within = np.arange(len(grp)) - gstart
    pos = run_off[s_core, s_bank, s_tile] + within
    gidx_all[s_core, pos] = s_bidx.astype(np.int16)
    drel_all[s_core, pos] = s_drel.astype(np.float32)

    meta = dict(K=K, chunks=chunks, batches=batches, bank_off=bank_off,
                nch_tot=nch_tot)
    return meta, gidx_all, drel_all


def wrap_idx(gidx_flat):
    """idx stream [E] -> dma_gather layout [128, E/16] (16-lane wrap,
    replicated into the 8 sixteen-partition groups)."""
    e = gidx_flat.shape[0]
    assert e % 16 == 0
    w = gidx_flat.reshape(e // 16, 16).T          # [16, E/16]
    return np.tile(w, (8, 1)).astype(np.int16)     # [128, E/16]


def prep_core_inputs(cfg, meta, core, x, W1, a_s1, a_d1, b1, W2, a_s2, a_d2,
                     b2, gidx_core, drel_core):
    n0, n1 = core * cfg.NPC, min((core + 1) * cfg.NPC, cfg.N)
    xs = np.zeros((cfg.NPAD, cfg.F_IN), np.float32)
    xs[: n1 - n0] = x[n0:n1]
    xT = np.ascontiguousarray(xs.T)                          # [F_IN, NPAD]
    kch = cfg.KCH
    xT_s = np.zeros((kch, P, cfg.NPAD), np.float32)
    for k in range(kch):
        lo, hi = k * P, min((k + 1) * P, cfg.F_IN)
        xT_s[k, : hi - lo] = xT[lo:hi]
    # [P, KCH, NPAD] so each node tile loads with ONE dma
    xT_s = np.ascontiguousarray(xT_s.transpose(1, 0, 2)).astype(BF16)

    A_s = np.zeros((cfg.D1, cfg.H1), np.float32)
    A_d = np.zeros((cfg.D1, cfg.H1), np.float32)
    for h in range(cfg.H1):
        A_s[h * cfg.O1:(h + 1) * cfg.O1, h] = a_s1[h]
        A_d[h * cfg.O1:(h + 1) * cfg.O1, h] = a_d1[h]
    # interleave: per head [8 msg cols | 1 zero col (1.0 memset later)]
    Wmsg = np.zeros((cfg.F_IN, cfg.F1), np.float32)
    for h in range(cfg.H1):
        Wmsg[:, h * (cfg.O1 + 1):h * (cfg.O1 + 1) + cfg.O1] = \
            W1[:, h * cfg.O1:(h + 1) * cfg.O1]
    Wfull = np.concatenate([Wmsg, W1 @ A_s, W1 @ A_d], axis=1)  # [F_IN, 88]
    wall = np.zeros((kch, P, cfg.WA1), np.float32)
    for k in range(kch):
        lo, hi = k * P, min((k + 1) * P, cfg.F_IN)
        wall[k, : hi - lo] = Wfull[lo:hi]
    wall = wall.astype(BF16)

    # layer 2: [W2 | zero col | W2@a_s2 | W2@a_d2]  (43 cols)
    w2aug = np.concatenate(
        [W2, np.zeros((cfg.D1, 1), np.float32),
         (W2 @ a_s2[0])[:, None], (W2 @ a_d2[0])[:, None]], axis=1
    ).astype(np.float32)

    bias1r = np.tile(b1[None, :], (P, 1)).astype(np.float32)
    bias2r = np.tile(b2[None, :], (P, 1)).astype(np.float32)
    iotar = np.tile(np.arange(P, dtype=np.float32)[None, :], (P, 1)).astype(BF16)
    iotap = np.arange(P, dtype=np.float32)[:, None].copy()
    ones1 = np.ones((1, P), BF16)
    identm = np.eye(P, dtype=np.float32)

    nch = meta["nch_tot"]
    gidx = wrap_idx(gidx_core)                               # [128, nch*8]
    drelc = np.ascontiguousarray(
        drel_core.reshape(nch, P).T).astype(np.float32)      # [128, nch]
    rows = []
    for b in range(cfg.NBANK):
        off = meta["bank_off"][b]
        for (lo, hi) in meta["batches"][b]:
            r = np.full((1, cfg.BATCH * P), -1.0, np.float32)
            r[0, : (hi - lo) * P] = drel_core[(off + lo) * P:(off + hi) * P]
            rows.append(r)
    drelf = (np.stack(rows).astype(BF16) if rows
             else np.zeros((1, 1, cfg.BATCH * P), BF16))

    return dict(xT=xT_s, wall=wall, w2aug=w2aug, bias1r=bias1r, bias2r=bias2r,
                iotar=iotar, iotap=iotap, ones1=ones1, identd=identm,
                gidx=gidx, drelc=drelc, drelf=drelf)


# ------------------------------------------------------------ bass program


def build_program(cfg, meta, phases="ABCDEFG"):
    nc = bacc.Bacc(None, target_bir_lowering=False, debug=False)
    f32, bf16, i16 = mybir.dt.float32, mybir.dt.bfloat16, mybir.dt.int16

    nch = meta["nch_tot"]
    nbatch_tot = sum(len(b) for b in meta["batches"])

    xT = nc.declare_dram_parameter("xT", [P, cfg.KCH, cfg.NPAD], bf16, isOutput=False)
    wall = nc.declare_dram_parameter("wall", [cfg.KCH, P, cfg.WA1], bf16, isOutput=False)
    w2aug = nc.declare_dram_parameter("w2aug", [cfg.D1, cfg.WA2], f32, isOutput=False)
    bias1r = nc.declare_dram_parameter("bias1r", [P, cfg.D1], f32, isOutput=False)
    bias2r = nc.declare_dram_parameter("bias2r", [P, cfg.C], f32, isOutput=False)
    identd = nc.declare_dram_parameter("identd", [P, P], f32, isOutput=False)
    iotar_d = nc.declare_dram_parameter("iotar", [P, P], bf16, isOutput=False)
    iotap_d = nc.declare_dram_parameter("iotap", [P, 1], f32, isOutput=False)
    ones1_d = nc.declare_dram_parameter("ones1", [1, P], bf16, isOutput=False)
    gidx_d = nc.declare_dram_parameter("gidx", [P, nch * 8], i16, isOutput=False)
    drelc_d = nc.declare_dram_parameter("drelc", [P, nch], f32, isOutput=False)
    drelf_d = nc.declare_dram_parameter("drelf", [nbatch_tot, 1, cfg.BATCH * P], bf16, isOutput=False)
    out_d = nc.declare_dram_parameter("out", [cfg.NPAD, cfg.C], f32, isOutput=True)

    t1loc = nc.dram_tensor("t1loc", [cfg.NPAD, cfg.ROW1], bf16)
    t1full = nc.dram_tensor("t1full", [cfg.NTOT, cfg.ROW1], bf16, addr_space="Shared")
    t2loc = nc.dram_tensor("t2loc", [cfg.NPAD, cfg.ROW2], bf16)
    t2full = nc.dram_tensor("t2full", [cfg.NTOT, cfg.ROW2], bf16, addr_space="Shared")

    H1, D1, O1, C = cfg.H1, cfg.D1, cfg.O1, cfg.C
    F1, F2 = cfg.F1, cfg.F2

    with TileContext(nc) as tc:
        with tc.tile_pool(name="persist", bufs=1) as pp:
            ident = pp.tile([P, P], f32)
            nc.sync.dma_start(out=ident[:], in_=identd[:])
            wall_sb = pp.tile([P, cfg.KCH, cfg.WA1], bf16)
            for k in range(cfg.KCH):
                nc.sync.dma_start(out=wall_sb[:, k, :], in_=wall[k])
            w2aug_sb = pp.tile([D1, cfg.WA2], f32)
            nc.sync.dma_start(out=w2aug_sb[:], in_=w2aug[:])
            b1_sb = pp.tile([P, D1], f32)
            nc.sync.dma_start(out=b1_sb[:], in_=bias1r[:])
            b2_sb = pp.tile([P, C], f32)
            nc.sync.dma_start(out=b2_sb[:], in_=bias2r[:])
            iotar_sb = pp.tile([P, P], bf16)
            nc.sync.dma_start(out=iotar_sb[:], in_=iotar_d[:])
            iotap_sb = pp.tile([P, 1], f32)
            nc.sync.dma_start(out=iotap_sb[:], in_=iotap_d[:])
            ones1_sb = pp.tile([1, P], bf16)
            nc.sync.dma_start(out=ones1_sb[:], in_=ones1_d[:])
            ad1_buf = pp.tile([P, cfg.TILES * H1], bf16)
            ad2_buf = pp.tile([P, cfg.TILES], bf16)
            agg1 = pp.tile([P, cfg.TILES * F1], f32)
            agg2 = pp.tile([P, cfg.TILES * F2], f32)
            ssum_all = pp.tile([P, cfg.TILES], f32)
            lns_all = pp.tile([P, cfg.TILES], f32)
            scr_g = pp.tile([1, cfg.ROW1], bf16)

            # ---------------- phase A: node transform layer 1
            if "A" in phases:
                with nc.named_scope("A_node1"):
                    node_phase1(nc, tc, cfg, xT, wall_sb, ident, ad1_buf, t1loc)

            # ---------------- phase B: allgather table 1
            # (single large AllGather: the collective cost is dominated by a
            # fixed small-transfer bandwidth floor, so splitting loses more
            # than bank-level overlap would gain)
            if "B" in phases:
                with nc.named_scope("B_ag1"):
                    nc.gpsimd.collective_compute(
                        "AllGather", mybir.AluOpType.bypass,
                        replica_groups=[list(range(NCORES))],
                        ins=[t1loc[:]], outs=[t1full[:]])

            # ---------------- phase C: edge layer 1
            if "C" in phases:
                with nc.named_scope("C_edge1"):
                    nc.vector.memset(agg1[:], 1e-16)
                    edge_phase(nc, tc, cfg, meta, layer=1, table=t1full,
                               row_elems=cfg.ROW1, fcols=F1, gidx_d=gidx_d,
                               drelc_d=drelc_d, drelf_d=drelf_d,
                               iotar_sb=iotar_sb, iotap_sb=iotap_sb,
                               ones1_sb=ones1_sb, scr_g=scr_g,
                               ad_buf=ad1_buf, agg=agg1)

            # ---------------- phase D: node transform layer 2
            if "D" in phases:
                with nc.named_scope("D_node2"):
                    node_phase2(nc, tc, cfg, agg1, b1_sb, w2aug_sb, ident,
                                ad2_buf, t2loc)

            # ---------------- phase E: allgather table 2
            if "E" in phases:
                with nc.named_scope("E_ag2"):
                    nc.gpsimd.collective_compute(
                        "AllGather", mybir.AluOpType.bypass,
                        replica_groups=[list(range(NCORES))],
                        ins=[t2loc[:]], outs=[t2full[:]])

            # ---------------- phase F: edge layer 2
            if "F" in phases:
                with nc.named_scope("F_edge2"):
                    nc.vector.memset(agg2[:], 1e-16)
                    edge_phase(nc, tc, cfg, meta, layer=2, table=t2full,
                               row_elems=cfg.ROW2, fcols=F2, gidx_d=gidx_d,
                               drelc_d=drelc_d, drelf_d=drelf_d,
                               iotar_sb=iotar_sb, iotap_sb=iotap_sb,
                               ones1_sb=ones1_sb, scr_g=scr_g,
                               ad_buf=ad2_buf, agg=agg2)

            # ---------------- phase G: epilogue (divide, bias, log_softmax)
            if "G" in phases:
                with nc.named_scope("G_epilogue"):
                    epilogue(nc, tc, cfg, agg2, b2_sb, out_d, ssum_all, lns_all)
            else:
                with tc.tile_pool(name="zz", bufs=1) as zz:
                    z = zz.tile([P, cfg.C], f32)
                    nc.vector.memset(z[:], 0.0)
                    for t in range(cfg.TILES):
                        nc.sync.dma_start(out=out_d[t * P:(t + 1) * P, :],
                                          in_=z[:])

    nc.compile()
    return nc


def node_phase1(nc, tc, cfg, xT, wall_sb, ident, ad1_buf, t1loc):
    f32, bf16 = mybir.dt.float32, mybir.dt.bfloat16
    H1, D1, F1, WA = cfg.H1, cfg.D1, cfg.F1, cfg.WA1
    with tc.tile_pool(name="na", bufs=3) as na, \
         tc.tile_pool(name="napsum", bufs=2, space="PSUM") as nap:
        for t in range(cfg.TILES):
            xt = na.tile([P, cfg.KCH, P], bf16, tag="xt")
            nc.sync.dma_start(out=xt[:], in_=xT[:, :, t * P:(t + 1) * P])
            row = na.tile([P, cfg.ROW1], bf16, tag="row")
            ph = nap.tile([WA, P], f32, tag="ph")
            for k in range(cfg.KCH):
                nc.tensor.matmul(out=ph[:], lhsT=wall_sb[:, k, :],
                                 rhs=xt[:, k, :],
                                 start=(k == 0), stop=(k == cfg.KCH - 1))
            hT = na.tile([WA, P], f32, tag="hT")
            nc.scalar.copy(out=hT[:], in_=ph[:])
            pr = nap.tile([P, WA], f32, tag="pr")
            nc.tensor.transpose(out=pr[:], in_=hT[:],
                                identity=ident[:WA, :WA])
            nc.vector.memset(row[:, WA:], 0.0)
            nc.scalar.copy(out=row[:, :WA], in_=pr[:])
            # denominator ride-along columns (head-interleaved col 8 of 9)
            nc.vector.memset(
                row[:, :F1].rearrange(
                    "p (h q) -> p h q", q=cfg.O1 + 1)[:, :, cfg.O1:], 1.0)
            nc.vector.tensor_copy(
                out=ad1_buf[:, t * H1:(t + 1) * H1],
                in_=pr[:, F1 + H1:F1 + 2 * H1])
            nc.scalar.dma_start(out=t1loc[t * P:(t + 1) * P, :], in_=row[:])


def node_phase2(nc, tc, cfg, agg1, b1_sb, w2aug_sb, ident, ad2_buf, t2loc):
    f32, bf16 = mybir.dt.float32, mybir.dt.bfloat16
    H1, D1, O1, C, F1, WA2 = cfg.H1, cfg.D1, cfg.O1, cfg.C, cfg.F1, cfg.WA2
    Q = O1 + 1
    with tc.tile_pool(name="nb", bufs=3) as nb, \
         tc.tile_pool(name="nbpsum", bufs=2, space="PSUM") as nbp:
        for t in range(cfg.TILES):
            a_t = agg1[:, t * F1:(t + 1) * F1].rearrange(
                "p (h q) -> p h q", q=Q)
            rec = nb.tile([P, H1], f32, tag="rec")
            nc.vector.reciprocal(
                out=rec[:].unsqueeze(2),
                in_=a_t[:, :, O1:])
            o1 = nb.tile([P, D1], f32, tag="o1")
            nc.vector.tensor_tensor(
                out=o1[:].rearrange("p (h o) -> p h o", h=H1),
                in0=a_t[:, :, :O1],
                in1=rec[:].unsqueeze(2).to_broadcast([P, H1, O1]),
                op=mybir.AluOpType.mult)
            nc.vector.tensor_add(out=o1[:], in0=o1[:], in1=b1_sb[:])
            # elu
            eneg = nb.tile([P, D1], f32, tag="eneg")
            nc.vector.tensor_scalar_min(eneg[:], o1[:], 0.0)
            nc.scalar.activation(out=eneg[:], in_=eneg[:],
                                 func=mybir.ActivationFunctionType.Exp)
            h = nb.tile([P, D1], f32, tag="h")
            nc.vector.tensor_scalar_max(h[:], o1[:], 0.0)
            nc.vector.tensor_add(out=h[:], in0=h[:], in1=eneg[:])
            nc.vector.tensor_scalar_add(h[:], h[:], -1.0)
            # h2 = [elu] @ w2aug via two PE transposes
            phT = nbp.tile([D1, P], f32, tag="phT")
            nc.tensor.transpose(out=phT[:], in_=h[:], identity=ident[:])
            hT2 = nb.tile([D1, P], f32, tag="hT2")
            nc.scalar.copy(out=hT2[:], in_=phT[:])
            p2T = nbp.tile([WA2, P], f32, tag="p2T")
            nc.tensor.matmul(out=p2T[:], lhsT=w2aug_sb[:], rhs=hT2[:],
                             start=True, stop=True)
            h2T = nb.tile([WA2, P], f32, tag="h2T")
            nc.scalar.copy(out=h2T[:], in_=p2T[:])
            p2 = nbp.tile([P, WA2], f32, tag="p2")
            nc.tensor.transpose(out=p2[:], in_=h2T[:],
                                identity=ident[:WA2, :WA2])
            row2 = nb.tile([P, cfg.ROW2], bf16, tag="row2")
            nc.vector.memset(row2[:, WA2:], 0.0)
            nc.scalar.copy(out=row2[:, :WA2], in_=p2[:])
            nc.vector.memset(row2[:, C:C + 1], 1.0)   # denominator column
            nc.vector.tensor_copy(out=ad2_buf[:, t:t + 1],
                                  in_=p2[:, WA2 - 1:WA2])
            nc.scalar.dma_start(out=t2loc[t * P:(t + 1) * P, :], in_=row2[:])


def epilogue(nc, tc, cfg, agg2, b2_sb, out_d, ssum_all, lns_all):
    f32 = mybir.dt.float32
    C, F2 = cfg.C, cfg.F2
    with tc.tile_pool(name="ep", bufs=3) as ep:
        # pass 1: o2 = agg2_msg * (1/denom) + bias, written in place into
        # agg2's msg columns; exp(o2) only to accumulate the softmax sums
        for t in range(cfg.TILES):
            rec = ep.tile([P, 1], f32, tag="rec2")
            nc.vector.reciprocal(
                out=rec[:], in_=agg2[:, t * F2 + C:t * F2 + C + 1])
            o2 = agg2[:, t * F2:t * F2 + C]
            nc.vector.scalar_tensor_tensor(
                out=o2, in0=o2, scalar=rec[:], in1=b2_sb[:],
                op0=mybir.AluOpType.mult, op1=mybir.AluOpType.add)
            exps = ep.tile([P, C], f32, tag="exps")
            nc.scalar.activation(out=exps[:], in_=o2,
                                 func=mybir.ActivationFunctionType.Exp,
                                 accum_out=ssum_all[:, t:t + 1])
        # pass 2: one Ln over all tiles, then subtract + store
        nc.scalar.activation(out=lns_all[:], in_=ssum_all[:],
                             func=mybir.ActivationFunctionType.Ln)
        for t in range(cfg.TILES):
            fin = ep.tile([P, C], f32, tag="fin")
            nc.vector.tensor_tensor(
                out=fin[:], in0=agg2[:, t * F2:t * F2 + C],
                in1=lns_all[:, t:t + 1].to_broadcast([P, C]),
                op=mybir.AluOpType.subtract)
            nc.scalar.dma_start(out=out_d[t * P:(t + 1) * P, :], in_=fin[:])


def edge_phase(nc, tc, cfg, meta, layer, table, row_elems, fcols, gidx_d,
               drelc_d, drelf_d, iotar_sb, iotap_sb, ones1_sb, scr_g,
               ad_buf, agg):
    f32, bf16, i16 = mybir.dt.float32, mybir.dt.bfloat16, mybir.dt.int16
    H = cfg.H1 if layer == 1 else 1
    Q = (cfg.O1 + 1) if layer == 1 else cfg.F2   # per-head block incl denom
    asl_lo = fcols                               # alpha_src col within row
    B = cfg.BATCH

    with tc.tile_pool(name=f"eg{layer}", bufs=3) as eg, \
         tc.tile_pool(name=f"es{layer}", bufs=2) as es, \
         tc.tile_pool(name=f"em{layer}", bufs=3) as em, \
         tc.tile_pool(name=f"epr{layer}", bufs=2, space="PSUM") as epr, \
         tc.tile_pool(name=f"epa{layer}", bufs=2, space="PSUM") as epa, \
         tc.tile_pool(name=f"epd{layer}", bufs=2, space="PSUM") as epd:
        psum_agg = None
        nb_seen = 0
        for b in range(cfg.NBANK):
            off = int(meta["bank_off"][b])
            tbl_bank = table[cfg.BANK_BASE[b]:
                             cfg.BANK_BASE[b] + cfg.BANK_ROWS[b], :]
            # gate this bank's gathers on its AllGather slice (Pool runs
            # in order, so later banks' gathers don't wait on earlier ones)
            nc.gpsimd.dma_start(out=scr_g[:], in_=tbl_bank[0:1, :])
            for (lo, hi) in meta["batches"][b]:
                nchb = hi - lo
                ngrp = nchb // 4
                idx_t = em.tile([P, B * 8], i16, tag="idx")
                nc.sync.dma_start(
                    out=idx_t[:, :nchb * 8],
                    in_=gidx_d[:, (off + lo) * 8:(off + hi) * 8])
                drc_t = em.tile([P, B], f32, tag="drc")
                nc.sync.dma_start(out=drc_t[:, :nchb],
                                  in_=drelc_d[:, off + lo:off + hi])
                drf_t = em.tile([1, B * P], bf16, tag="drf")
                nc.sync.dma_start(out=drf_t[:], in_=drelf_d[nb_seen])
                g = eg.tile([P, B, row_elems], bf16, tag="g")
                nc.gpsimd.memset(g[0:1, 0:1, 0:4], 0.0)   # hoist WAR dep
                nc.gpsimd.dma_gather(
                    out_ap=g[:, :nchb, :], in_ap=tbl_bank,
                    idxs_ap=idx_t[:, :nchb * 8], num_idxs=nchb * P,
                    num_idxs_reg=nchb * P, elem_size=row_elems,
                    single_packet=False)
                nb_seen += 1

                # per-batch tiles: scatter masks + edge weights
                st_all = es.tile([P, B, P], bf16, tag="st")
                w_all = em.tile([P, B, H], f32, tag="w")
                wmx = em.tile([P, B, H], f32, tag="wmx")
                pad_all = epd.tile([P, B * H], f32, tag="pad")

                for gi in range(ngrp):
                    c0 = 4 * gi
                    pr1 = epr.tile([P, 512], f32, tag="pr1")
                    nc.tensor.matmul(out=pr1[:], lhsT=ones1_sb[:],
                                     rhs=drf_t[0:1, gi * 512:(gi + 1) * 512],
                                     start=True, stop=True)
                    spre = em.tile([P, 512], bf16, tag="spre")
                    nc.scalar.copy(out=spre[:], in_=pr1[:])
                    s = em.tile([P, 4, P], bf16, tag="s")
                    nc.vector.tensor_scalar(
                        out=s[:].rearrange("p a b -> p (a b)"),
                        in0=spre[:], scalar1=iotap_sb[:], scalar2=None,
                        op0=mybir.AluOpType.is_equal)
                    for c in range(4):
                        nc.vector.tensor_scalar(
                            out=st_all[:, c0 + c, :], in0=iotar_sb[:],
                            scalar1=drc_t[:, c0 + c:c0 + c + 1], scalar2=None,
                            op0=mybir.AluOpType.is_equal)
                        t_c = meta["chunks"][b][lo + c0 + c][0]
                        nc.tensor.matmul(
                            out=pad_all[:, (c0 + c) * H:(c0 + c + 1) * H],
                            lhsT=s[:, c, :],
                            rhs=ad_buf[:, t_c * H:(t_c + 1) * H],
                            start=True, stop=True)

                # batched edge-weight pipeline: e = as + ad; exp(leakyrelu(e))
                nc.vector.tensor_tensor(
                    out=w_all[:, :nchb, :],
                    in0=g[:, :nchb, asl_lo:asl_lo + H],
                    in1=pad_all[:, :nchb * H].rearrange(
                        "p (a h) -> p a h", h=H),
                    op=mybir.AluOpType.add)
                nc.vector.scalar_tensor_tensor(
                    out=wmx[:, :nchb, :], in0=w_all[:, :nchb, :],
                    scalar=cfg.NEG, in1=w_all[:, :nchb, :],
                    op0=mybir.AluOpType.mult, op1=mybir.AluOpType.max)
                nc.scalar.activation(
                    out=w_all[:, :nchb, :], in_=wmx[:, :nchb, :],
                    func=mybir.ActivationFunctionType.Exp)

                for gi in range(ngrp):
                    c0 = 4 * gi
                    m = em.tile([P, 4, fcols], bf16, tag="m")
                    nc.vector.tensor_tensor(
                        out=m[:].rearrange("p a (h q) -> p a h q", q=Q),
                        in0=g[:, c0:c0 + 4, :fcols].rearrange(
                            "p a (h q) -> p a h q", q=Q),
                        in1=w_all[:, c0:c0 + 4, :].unsqueeze(3).to_broadcast(
                            [P, 4, H, Q]),
                        op=mybir.AluOpType.mult)
                    for c in range(4):
                        t_c, start_c, stop_c = meta["chunks"][b][lo + c0 + c]
                        if start_c:
                            psum_agg = epa.tile([P, fcols], f32, tag="agg")
                        nc.tensor.matmul(out=psum_agg[:],
                                         lhsT=st_all[:, c0 + c, :],
                                         rhs=m[:, c, :],
                                         start=start_c, stop=stop_c)
                        if stop_c:
                            nc.vector.tensor_tensor(
                                out=agg[:, t_c * fcols:(t_c + 1) * fcols],
                                in0=agg[:, t_c * fcols:(t_c + 1) * fcols],
                                in1=psum_agg[:], op=mybir.AluOpType.add)


# ------------------------------------------------------------------ kernel

_CACHE = {}


def kernel(**inputs):
    cfg = FULL
    x = np.asarray(inputs["x"], np.float32)
    ei = np.asarray(inputs["edge_index"])
    W1 = np.asarray(inputs["W1"], np.float32)
    a_s1 = np.asarray(inputs["att_src1"], np.float32)
    a_d1 = np.asarray(inputs["att_dst1"], np.float32)
    b1 = np.asarray(inputs["bias1"], np.float32)
    W2 = np.asarray(inputs["W2"], np.float32)
    a_s2 = np.asarray(inputs["att_src2"], np.float32)
    a_d2 = np.asarray(inputs["att_dst2"], np.float32)
    b2 = np.asarray(inputs["bias2"], np.float32)

    loops = np.arange(cfg.N, dtype=np.int64)
    src = np.concatenate([ei[0].astype(np.int64), loops])
    dst = np.concatenate([ei[1].astype(np.int64), loops])

    meta, gidx_all, drel_all = build_edge_meta(cfg, src, dst)

    schedule_sig = tuple(
        tuple(meta["chunks"][b]) for b in range(cfg.NBANK))
    key = ("full", meta["nch_tot"], hash(schedule_sig))
    if key not in _CACHE:
        _CACHE[key] = build_program(cfg, meta)
    nc = _CACHE[key]

    in_maps = []
    for c in range(NCORES):
        in_maps.append(prep_core_inputs(
            cfg, meta, c, x, W1, a_s1, a_d1, b1, W2, a_s2, a_d2, b2,
            gidx_all[c], drel_all[c]))
    res = run_bass_kernel_spmd(nc, in_maps, list(range(NCORES)))
    outs = [res.results[c]["out"][: cfg.NPC] for c in range(NCORES)]
    return np.concatenate(outs, axis=0)[: cfg.N].astype(np.float32)
